# revision 33
# baseline (speedup 1.0000x reference)
"""Trainium2 Bass kernel for nn_BaseDependentAttentionLayer (GNN edge attention).

Strategy (8 NeuronCores):
  - Shard origin nodes contiguously: core r owns origins [1250r, 1250(r+1)).
    Host sorts edges by origin so segment-softmax/scatter are core-local.
  - LayerNorm affine + attention scale folded into QKV weights on host;
    MLP collapsed (W12 = W1@W2).
  - Phase A: LN (bf16) + QKV per 128-node window; k then v AllGathered
    (bf16, Shared-HBM output => local writes + barrier on real HW) into
    full [N, 512] tables; q kept local.
  - Phase B per 128-origin window: one-hot "transpose-select" matmuls
    (is_transpose=True with a one-hot-column moving matrix) broadcast
    q-rows to edge columns and exp-rows to 64-wide head blocks, writing
    bf16 directly to PSUM (validated on HW) — keeps DVE elementwise in 2x
    mode and avoids scalar-engine copies. Scores reduce per head via a
    one-hot hmask matmul; scatter-add + denominator are one-hot matmuls
    accumulating over the window in PSUM. Per-window tile counts.
  - Softmax without max-subtraction (scores are O(1), exp safe in fp32);
    normalization after the scatter.
"""

import sys

sys.path.insert(0, "/opt/trn_rl_repo")

import numpy as np
import ml_dtypes

bf16 = ml_dtypes.bfloat16

N, E, D, H = 10000, 160000, 512, 8
HD = D // H
SCALE = HD**-0.5
NCORES = 8
NPC = N // NCORES  # 1250 nodes/core
W = 10  # windows per core
WIN = 128  # origins per window
ET = 128  # edges per tile
EPS_LN = 1e-5
EPS_DEN = 1e-16


def _host_prep(origin, dest, ew):
    order = np.argsort(origin, kind="stable")
    o_s, d_s = origin[order], dest[order]
    core_of = o_s // NPC
    wloc = (o_s - core_of * NPC) // WIN
    counts = np.zeros((NCORES, W), np.int64)
    for r in range(NCORES):
        cm = core_of == r
        wl = wloc[cm]
        for w in range(W):
            counts[r, w] = int(np.sum(wl == w))
    Tw = [max(1, int(np.ceil(counts[:, w].max() / ET))) for w in range(W)]
    T = max(Tw)
    cofs = np.array([w * T * ET for w in range(W + 1)])
    NB = int(cofs[-1])
    percore = []
    for r in range(NCORES):
        dd_ = np.zeros(NB, np.int16)  # global dest node id
        ol = np.zeros(NB, np.int64)  # origin-local (0..127); pads 0
        vm = np.zeros(NB, bool)  # valid (non-pad)
        et = np.zeros((NB, H), np.float32)
        cm = core_of == r
        for w in range(W):
            m = cm & (wloc == w)
            cnt = int(m.sum())
            o = int(cofs[w])
            dd_[o:o + cnt] = d_s[m].astype(np.int16)
            ol[o:o + cnt] = o_s[m] - r * NPC - w * WIN
            vm[o:o + cnt] = True
            et[o:o + cnt] = ew[order[m]]
        percore.append(dict(dd=dd_, ol=ol, vm=vm, et=et))
    return percore, Tw, cofs


def _wrap_idx(idx_flat):
    """int16 [n] -> wrapped [128, n/16] layout for dma_gather (idx i at
    [i%16, i//16], replicated over the 8 Q7 partition groups)."""
    w = idx_flat.reshape(-1, 16).T  # [16, n/16]
    return np.tile(w, (8, 1)).astype(np.int16)


def _win_geometry(Tw):
    """Per-window halves (gather granularity) and 4-tile blocks."""
    geo = []
    for T in Tw:
        hn0 = min(T, ((T + 1) // 2 + 3) // 4 * 4)
        halves = [(0, hn0)] + ([(hn0, T - hn0)] if T > hn0 else [])
        blocks = []
        for h0, hn in halves:
            for b0 in range(h0, h0 + hn, 4):
                blocks.append((b0, min(4, h0 + hn - b0)))
        geo.append((halves, blocks))
    return geo


def _build_program(Tw, mock_ag=False, use_bias=True):
    import concourse.bass as bass
    import concourse.bacc as bacc
    import concourse.mybir as mybir
    import concourse.tile as tile

    dt = mybir.dt
    Alu = mybir.AluOpType
    Act = mybir.ActivationFunctionType

    TM = max(Tw)
    NB = W * TM * ET
    cofs = [w * TM * ET for w in range(W + 1)]
    GEO = _win_geometry(Tw)

    nc = bacc.Bacc(
        "TRN2", target_bir_lowering=False, debug=False, num_devices=NCORES
    )

    # ---------------- I/O ----------------
    xsb_t = nc.dram_tensor("xsb", [W * 128, D], dt.bfloat16, kind="ExternalInput")
    wq_t = nc.dram_tensor("wq", [128, 4, D], dt.bfloat16, kind="ExternalInput")
    wk_t = nc.dram_tensor("wk", [128, 4, D], dt.bfloat16, kind="ExternalInput")
    wv_t = nc.dram_tensor("wv", [128, 4, D], dt.bfloat16, kind="ExternalInput")
    w12_t = nc.dram_tensor("w12", [128, 4, D], dt.bfloat16, kind="ExternalInput")
    bias_t = nc.dram_tensor("bias", [1, 4, D], dt.bfloat16, kind="ExternalInput")
    hmask_t = nc.dram_tensor("hmask", [128, 4, H], dt.bfloat16, kind="ExternalInput")
    m1w_t = nc.dram_tensor("m1w", [H, D], dt.bfloat16, kind="ExternalInput")
    ones_t = nc.dram_tensor("ones1", [1, 128], dt.bfloat16, kind="ExternalInput")
    ident_t = nc.dram_tensor("ident", [128, 128], dt.bfloat16, kind="ExternalInput")
    ident8_t = nc.dram_tensor("ident8", [128, 128], dt.float8e4, kind="ExternalInput")
    dw_t = nc.dram_tensor("dw", [128, NB // 16], dt.int16, kind="ExternalInput")
    st_t = nc.dram_tensor("st", [128, NB], dt.bfloat16, kind="ExternalInput")
    stt_t = nc.dram_tensor("stt", [128, NB], dt.float8e4, kind="ExternalInput")
    ewt_t = nc.dram_tensor("ewt", [H, NB], dt.bfloat16, kind="ExternalInput")
    out_t = nc.dram_tensor("out", [W * 128, D], dt.bfloat16, kind="ExternalOutput")

    with tile.TileContext(nc) as tc:
        with (
            tc.tile_pool(name="const", bufs=1) as cpool,
            tc.tile_pool(name="persist", bufs=1) as ppool,
            tc.tile_pool(name="dram", bufs=1, space="DRAM") as dpool,
        ):
            # constants
            wq = cpool.tile([128, 4, D], dt.bfloat16)
            wk = cpool.tile([128, 4, D], dt.bfloat16)
            wv = cpool.tile([128, 4, D], dt.bfloat16)
            w12 = cpool.tile([128, 4, D], dt.bfloat16)
            biases = cpool.tile([1, 4, D], dt.bfloat16)
            hmask = cpool.tile([128, 4, H], dt.bfloat16)
            m1w = cpool.tile([H, D], dt.bfloat16)
            ones1 = cpool.tile([1, 128], dt.bfloat16)
            ident = cpool.tile([128, 128], dt.bfloat16)
            ident8 = cpool.tile([128, 128], dt.float8e4)
            dw = cpool.tile([128, NB // 16], dt.int16)
            for tl, tn in [
                (wq, wq_t), (wk, wk_t), (wv, wv_t), (w12, w12_t),
                (biases, bias_t), (hmask, hmask_t), (m1w, m1w_t),
                (ones1, ones_t), (ident, ident_t), (ident8, ident8_t), (dw, dw_t),
            ]:
                nc.sync.dma_start(tl[:], tn.ap())

            # persistent activations
            q_sb = ppool.tile([128, W, D], dt.bfloat16)
            xg_all = ppool.tile([128, W, D], dt.bfloat16)
            zT_all = ppool.tile([128, W, 4, 128], dt.bfloat16)
            values = ppool.tile([128, W, D], dt.bfloat16)
            vT = ppool.tile([128, 4, W, 128], dt.bfloat16)

            # collective buffers (Shared: all 8 cores on one device's HBM)
            k_in = dpool.tile([NPC, D], dt.bfloat16)
            v_in = dpool.tile([NPC, D], dt.bfloat16)
            k_full = dpool.tile([N, D], dt.bfloat16, addr_space="Shared")
            v_full = dpool.tile([N, D], dt.bfloat16, addr_space="Shared")

            # ---------------- Phase A: LN + QKV ----------------
            pb_ctx = tc.tile_pool(name="pB", bufs=2)
            pb = pb_ctx.__enter__()
            TMET = TM * ET

            def loadw(w):
                co = cofs[w]
                T = Tw[w]
                stw = pb.tile([128, TMET], dt.bfloat16, tag="stw")
                nc.sync.dma_start(stw[:, :T * ET], st_t.ap()[:, co:co + T * ET])
                sttw = pb.tile([128, TMET], dt.float8e4, tag="sttw")
                nc.sync.dma_start(sttw[:, :T * ET], stt_t.ap()[:, co:co + T * ET])
                ewtw = pb.tile([H, TMET], dt.bfloat16, tag="ewtw")
                nc.sync.dma_start(ewtw[:, :T * ET], ewt_t.ap()[:, co:co + T * ET])
                return stw, sttw, ewtw

            preloaded = [loadw(0), loadw(1)]
            with (
                nc.named_scope("phaseA_qkv"),
                tc.tile_pool(name="pA", bufs=4) as pa,
                tc.tile_pool(name="psA", bufs=3, space="PSUM") as psa,
            ):
                def proj(g, wt, bi, dst_sb, dram=None):
                    lo = g * 128
                    rows = min(128, NPC - lo)
                    ps = psa.tile([128, D], dt.float32, tag="qkv_ps")
                    for c in range(4):
                        nc.tensor.matmul(
                            ps[:], zT_all[:, g, c, :], wt[:, c, :],
                            start=(c == 0), stop=(c == 3 and not use_bias),
                        )
                    if use_bias:
                        nc.tensor.matmul(
                            ps[:], ones1[:], biases[:, bi, :], start=False, stop=True
                        )
                    if dst_sb is None:
                        kvt = pa.tile([128, D], dt.bfloat16, tag="kvt")
                        if dram is v_in:
                            nc.vector.tensor_copy(kvt[:], ps[:])
                        else:
                            nc.scalar.copy(kvt[:], ps[:])
                        nc.sync.dma_start(dram[lo:lo + rows, :], kvt[:rows, :])
                    else:
                        nc.scalar.copy(dst_sb[:], ps[:])

                def kproj(g):
                    proj(g, wk, 1, None, k_in)

                def vproj(g):
                    proj(g, wv, 2, None, v_in)

                for g in range(W):
                    xg = xg_all[:, g, :]
                    nc.sync.dma_start(xg, xsb_t.ap()[g * 128:(g + 1) * 128, :])
                    musum = pa.tile([128, 1], dt.float32, tag="musum")
                    nc.vector.tensor_reduce(musum[:], xg, mybir.AxisListType.X, Alu.add)
                    mu = pa.tile([128, 1], dt.float32, tag="mu")
                    nc.vector.tensor_scalar_mul(mu[:], musum[:], 1.0 / D)
                    xc = pa.tile([128, D], dt.bfloat16, tag="xc")
                    nc.vector.tensor_scalar(xc[:], xg, mu[:], None, Alu.subtract)
                    sq = pa.tile([128, D], dt.bfloat16, tag="sq")
                    vs = pa.tile([128, 1], dt.float32, tag="vs")
                    nc.vector.scalar_tensor_tensor(
                        sq[:], xc[:], 1.0, xc[:], Alu.bypass, Alu.mult, accum_out=vs[:]
                    )
                    vr = pa.tile([128, 1], dt.float32, tag="vr")
                    nc.vector.tensor_scalar(vr[:], vs[:], 1.0 / D, EPS_LN, Alu.mult, Alu.add)
                    sd = pa.tile([128, 1], dt.float32, tag="sd")
                    nc.scalar.sqrt(sd[:], vr[:])
                    rstd = pa.tile([128, 1], dt.float32, tag="rstd")
                    nc.vector.reciprocal(rstd[:], sd[:])
                    z = pa.tile([128, D], dt.bfloat16, tag="z")
                    nc.vector.tensor_scalar(z[:], xc[:], rstd[:], None, Alu.mult)
                    zT_ps = psa.tile([128, 4, 128], dt.bfloat16, tag="zT_ps")
                    for c in range(4):
                        nc.tensor.transpose(
                            zT_ps[:, c, :], z[:, c * 128:(c + 1) * 128], ident[:]
                        )
                    nc.scalar.copy(zT_all[:, g, :, :], zT_ps[:])
                    kproj(g)
                    vproj(g)

                if mock_ag:
                    nc.sync.dma_start(k_full[0:NPC, :], k_in[:])
                else:
                    nc.gpsimd.collective_compute(
                        "AllGather", Alu.bypass,
                        replica_groups=[list(range(NCORES))],
                        ins=[k_in.opt()], outs=[k_full.opt()],
                    )
                if mock_ag:
                    nc.sync.dma_start(v_full[0:NPC, :], v_in[:])
                else:
                    nc.gpsimd.collective_compute(
                        "AllGather", Alu.bypass,
                        replica_groups=[list(range(NCORES))],
                        ins=[v_in.opt()], outs=[v_full.opt()],
                    )
                for g in range(W):
                    proj(g, wq, 0, q_sb[:, g, :])

            # ---------------- Phase B: edge loop ----------------
            with (
                nc.named_scope("phaseB_edges"),
                tc.tile_pool(name="psSel", bufs=1, space="PSUM") as pssel,
                tc.tile_pool(name="psSc", bufs=1, space="PSUM") as pssc,
                tc.tile_pool(name="psAcc", bufs=2, space="PSUM") as psacc,
                tc.tile_pool(name="psDen", bufs=1, space="PSUM") as psden,
            ):
                ghalves = _win_geometry([TM])[0][0]
                pending = []
                for w in range(W):
                    T = Tw[w]
                    _, blocks = GEO[w]
                    co = cofs[w]
                    gh = []
                    for h0, hn in ghalves:
                        ni = hn * ET
                        c0 = (co + h0 * ET) // 16
                        kT = pb.tile([128, 4, ni], dt.bfloat16, tag=f"kT{h0 > 0}")
                        nc.gpsimd.dma_gather(
                            out_ap=kT[:], in_ap=k_full[:],
                            idxs_ap=dw[:, c0:c0 + ni // 16],
                            num_idxs=ni, num_idxs_reg=ni, elem_size=D,
                            transpose=True, single_packet=False,
                        )
                        vG = pb.tile([128, hn, D], dt.bfloat16, tag=f"vG{h0 > 0}")
                        nc.gpsimd.dma_gather(
                            out_ap=vG[:], in_ap=v_full[:],
                            idxs_ap=dw[:, c0:c0 + ni // 16],
                            num_idxs=ni, num_idxs_reg=ni, elem_size=D,
                            single_packet=False,
                        )
                        gh.append((kT, vG))

                    stw, sttw, ewtw = preloaded[w] if w < 2 else loadw(w)

                    unnorm = psacc.tile([128, D], dt.float32, tag="unnorm")
                    denomB = psden.tile([128, H], dt.float32, tag="denomB")

                    def stage1(t0, bt):
                        EB = bt * ET
                        ecol = t0 * ET
                        hf = 0 if t0 < ghalves[0][1] else 1
                        kT, vG = gh[hf]
                        h0 = ghalves[hf][0]
                        kcol = (t0 - h0) * ET
                        # q broadcast to edge cols: bf16 PSUM via select
                        qgT = pssel.tile([128, 4, 512], dt.bfloat16, tag="qgT")
                        for c in range(4):
                            nc.tensor.transpose(
                                qgT[:, c, :EB],
                                q_sb[:, w, c * 128:(c + 1) * 128],
                                stw[:, ecol:ecol + EB],
                            )
                        kq = pb.tile([128, 4, 512], dt.bfloat16, tag="kq")
                        nc.vector.tensor_tensor(
                            kq[:, :, :EB], kT[:, :, kcol:kcol + EB],
                            qgT[:, :, :EB], Alu.mult,
                        )
                        return (t0, bt, kq, vG, h0)

                    def stage2(st1):
                        t0, bt, kq, vG, h0 = st1
                        EB = bt * ET
                        ecol = t0 * ET
                        sc = pssc.tile([8, 512], dt.float32, tag="sc")
                        for c in range(4):
                            nc.tensor.matmul(
                                sc[:, :EB], hmask[:, c, :], kq[:, c, :EB],
                                start=(c == 0), stop=(c == 3),
                            )
                        ws = pb.tile([8, 512], dt.bfloat16, tag="ws")
                        nc.vector.tensor_tensor(
                            ws[:, :EB], sc[:, :EB], ewtw[:, ecol:ecol + EB], Alu.mult
                        )
                        ews = pb.tile([8, 512], dt.bfloat16, tag="ews")
                        nc.scalar.activation(ews[:, :EB], ws[:, :EB], Act.Exp)
                        # exp broadcast to 64-wide head blocks: bf16 PSUM
                        b_ps = pssel.tile([128, 4, D], dt.bfloat16, tag="b_ps")
                        for t in range(bt):
                            nc.tensor.transpose(
                                b_ps[:, t, :], ews[:, t * ET:(t + 1) * ET], m1w[:]
                            )
                        wvx = pb.tile([128, 4, D + H], dt.bfloat16, tag="wvx")
                        nc.vector.tensor_tensor(
                            wvx[:, :bt, :D], vG[:, t0 - h0:t0 - h0 + bt, :],
                            b_ps[:, :bt, :D], Alu.mult,
                        )
                        nc.scalar.copy(wvx[:, :bt, D:D + H], b_ps[:, :bt, ::HD])
                        for t in range(bt):
                            tt = t0 + t
                            stcol = tt * ET
                            nc.tensor.matmul(
                                unnorm[:], sttw[:, stcol:stcol + ET], wvx[:, t, :D],
                                start=(tt == 0), stop=(tt == T - 1),
                            )
                            nc.tensor.matmul(
                                denomB[:], sttw[:, stcol:stcol + ET], wvx[:, t, D:D + H],
                                start=(tt == 0), stop=(tt == T - 1),
                            )

                    st1 = None
                    for t0, bt in blocks:
                        cur = stage1(t0, bt)
                        if st1 is not None:
                            stage2(st1)
                        st1 = cur
                    stage2(st1)

                    # window epilogue (emitted one window later for
                    # PE-queue pipelining): divide + transpose + MLP
                    def epilogue(w, unnorm, denomB):
                        den8 = pb.tile([128, H], dt.float32, tag="den8")
                        nc.vector.tensor_scalar(den8[:], denomB[:], EPS_DEN, None, Alu.add)
                        rec8 = pb.tile([128, H], dt.float32, tag="rec8")
                        nc.vector.reciprocal(rec8[:], den8[:])
                        un_sb = pb.tile([128, D], dt.float32, tag="un_sb")
                        nc.scalar.copy(un_sb[:], unnorm[:])
                        for h in range(H):
                            nc.gpsimd.tensor_scalar(
                                values[:, w, h * HD:(h + 1) * HD],
                                un_sb[:, h * HD:(h + 1) * HD],
                                rec8[:, h:h + 1], None, Alu.mult,
                            )
                        vt_ps = pssel.tile([128, 4, 512], dt.bfloat16, tag="qgT")
                        for c in range(4):
                            nc.tensor.transpose(
                                vt_ps[:, c, :128], values[:, w, c * 128:(c + 1) * 128], ident[:]
                            )
                        nc.vector.tensor_copy(vT[:, :, w, :], vt_ps[:, :, :128])
                        mlp_ps = psacc.tile([128, D], dt.float32, tag="unnorm")
                        for c in range(4):
                            nc.tensor.matmul(
                                mlp_ps[:], vT[:, c, w, :], w12[:, c, :],
                                start=(c == 0), stop=(c == 3 and not use_bias),
                            )
                        if use_bias:
                            nc.tensor.matmul(
                                mlp_ps[:], ones1[:], biases[:, 3, :], start=False, stop=True
                            )
                        og = pb.tile([128, D], dt.bfloat16, tag="og")
                        nc.vector.tensor_tensor(og[:], mlp_ps[:], xg_all[:, w, :], Alu.add)
                        nc.sync.dma_start(out_t.ap()[w * 128:(w + 1) * 128, :], og[:])

                    pending.append((w, unnorm, denomB))
                    if len(pending) > 1:
                        epilogue(*pending.pop(0))
                for args in pending:
                    epilogue(*args)

            pb_ctx.__exit__(None, None, None)

    nc.compile()
    from concourse.bass_interp import get_hw_module

    nc.m = get_hw_module(nc.m)
    return nc


def kernel(x, edge_index, edge_weights, ln_g, ln_b, Wq, bq, Wk, bk, Wv, bv,
           W1, b1, W2, b2, _trace=False):
    x = np.asarray(x, np.float32)
    ei = np.asarray(edge_index)
    ew = np.asarray(edge_weights, np.float32)
    origin, dest = ei[0].astype(np.int64), ei[1].astype(np.int64)

    percore, Tw, cofs = _host_prep(origin, dest, ew)

    # fold LN affine + attention scale into weights (host, fp32)
    ln_g = np.asarray(ln_g, np.float32)
    ln_b = np.asarray(ln_b, np.float32)
    Wq_f = (ln_g[:, None] * np.asarray(Wq, np.float32)) * SCALE
    bq_f = (ln_b @ np.asarray(Wq, np.float32)) * SCALE + np.asarray(bq, np.float32) * SCALE
    Wk_f = ln_g[:, None] * np.asarray(Wk, np.float32)
    bk_f = ln_b @ np.asarray(Wk, np.float32) + np.asarray(bk, np.float32)
    Wv_f = ln_g[:, None] * np.asarray(Wv, np.float32)
    bv_f = ln_b @ np.asarray(Wv, np.float32) + np.asarray(bv, np.float32)
    W12 = np.asarray(W1, np.float32) @ np.asarray(W2, np.float32)
    b12 = np.asarray(b1, np.float32) @ np.asarray(W2, np.float32) + np.asarray(b2, np.float32)

    f8 = ml_dtypes.float8_e4m3

    def chunked(wm):  # [512, 512] -> [128, 4, 512] bf16
        return np.ascontiguousarray(
            wm.reshape(4, 128, D).transpose(1, 0, 2)
        ).astype(bf16)

    def chunked8(wm):  # [512, 512] -> [128, 2, 2, 512] fp8 DoubleRow layout
        return np.ascontiguousarray(
            wm.reshape(2, 2, 128, D).transpose(2, 0, 1, 3)
        ).astype(f8)

    hmask = np.zeros((128, 4, H), np.float32)
    for c in range(4):
        for d in range(128):
            hmask[d, c, (128 * c + d) // HD] = 1.0
    m1w = np.zeros((H, D), np.float32)
    for h in range(H):
        m1w[h, h * HD:(h + 1) * HD] = 1.0
    bias_all = np.stack([bq_f, bk_f, bv_f, b12])[None]  # [1, 4, 512]

    common = dict(
        wq=chunked(Wq_f), wk=chunked(Wk_f), wv=chunked(Wv_f), w12=chunked(W12),
        bias=bias_all.astype(bf16), hmask=hmask.astype(bf16), m1w=m1w.astype(bf16),
        ones1=np.ones((1, 128), bf16),
        ident=np.eye(128, dtype=bf16),
        ident8=np.eye(128, dtype=ml_dtypes.float8_e4m3),
    )

    NB = int(cofs[-1])
    in_maps = []
    ar = np.arange(NB)
    tile_of = ar // ET
    e_in_tile = ar % ET
    for r in range(NCORES):
        pc = percore[r]
        ol = pc["ol"]
        vm = pc["vm"]
        st = np.zeros((128, NB), bf16)
        st[ol, ar] = 1
        stt = np.zeros((128, NB), ml_dtypes.float8_e4m3)
        stt[e_in_tile[vm], tile_of[vm] * ET + ol[vm]] = 1
        xsb = np.zeros((W * 128, D), np.float32)
        xsb[:NPC] = x[r * NPC:(r + 1) * NPC]
        in_maps.append(dict(
            xsb=xsb.astype(bf16),
            xs=xsb,
            dw=_wrap_idx(pc["dd"]),
            st=st, stt=stt,
            ewt=np.ascontiguousarray(pc["et"].T).astype(bf16),
            **common,
        ))

    use_bias = any(
        float(np.abs(b).max()) > 0 for b in (bq_f, bk_f, bv_f, b12)
    )
    nc = _build_program(Tw, use_bias=use_bias)
    from concourse import bass_utils

    res = bass_utils.run_bass_kernel_spmd(
        nc, in_maps, core_ids=list(range(NCORES))
    )
    out = np.concatenate(
        [res.results[r]["out"][:NPC] for r in range(NCORES)], axis=0
    )
    kernel.last_result = res
    if _trace:
        import bench_hw

        kernel.exec_time_ns = bench_hw.bench(nc, in_maps, NCORES)
    return out.astype(np.float32)


# revision 63
# speedup vs baseline: 2.1229x; 2.1229x over previous
"""Trainium2 Bass kernel for nn_BaseDependentAttentionLayer (GNN edge attention).

Strategy (8 NeuronCores):
  - Shard origin nodes contiguously: core r owns origins [1250r, 1250(r+1)).
    Host sorts edges by origin so segment-softmax/scatter are core-local.
  - LayerNorm affine + attention scale folded into QKV weights on host;
    MLP collapsed (W12 = W1@W2).
  - Phase A: LN (bf16) + QKV per 128-node window; k then v AllGathered
    (bf16, Shared-HBM output => local writes + barrier on real HW) into
    full [N, 512] tables; q kept local.
  - Phase B per 128-origin window: one-hot "transpose-select" matmuls
    (is_transpose=True with a one-hot-column moving matrix) broadcast
    q-rows to edge columns and exp-rows to 64-wide head blocks, writing
    bf16 directly to PSUM (validated on HW) — keeps DVE elementwise in 2x
    mode and avoids scalar-engine copies. Scores reduce per head via a
    one-hot hmask matmul; scatter-add + denominator are one-hot matmuls
    accumulating over the window in PSUM. Per-window tile counts.
  - Softmax without max-subtraction (scores are O(1), exp safe in fp32);
    normalization after the scatter.
"""

import sys

sys.path.insert(0, "/opt/trn_rl_repo")

import numpy as np
import ml_dtypes

bf16 = ml_dtypes.bfloat16

N, E, D, H = 10000, 160000, 512, 8
HD = D // H
SCALE = HD**-0.5
NCORES = 8
NPC = N // NCORES  # 1250 nodes/core
W = 10  # windows per core
WIN = 128  # origins per window
ET = 128  # edges per tile
EPS_LN = 1e-5
EPS_DEN = 1e-16


def _host_prep(origin, dest, ew):
    # per-core LPT balancing: permute each core's origins across windows so
    # window edge-counts even out (smaller max tile count T). perm[r][slot]
    # = local origin id occupying window slot (w = slot//128).
    deg = np.bincount(origin, minlength=N)
    perms = []
    counts = np.zeros((NCORES, W), np.int64)
    for r in range(NCORES):
        dloc = deg[r * NPC:(r + 1) * NPC]
        order_by_deg = np.argsort(-dloc, kind="stable")
        wsum = np.zeros(W, np.int64)
        wfill = np.zeros(W, np.int64)
        perm = np.full(NPC, -1, np.int64)
        slots_per_w = [min(128, NPC - w * 128) for w in range(W)]
        for o in order_by_deg:
            cands = [w for w in range(W) if wfill[w] < slots_per_w[w]]
            w = min(cands, key=lambda x: (wsum[x], x))
            perm[w * 128 + wfill[w]] = o
            wsum[w] += dloc[o]
            wfill[w] += 1
        counts[r] = wsum
        perms.append(perm)
    Tw = [max(1, int(np.ceil(counts[:, w].max() / ET))) for w in range(W)]
    T = max(Tw)
    cofs = np.array([w * T * ET for w in range(W + 1)])
    NB = int(cofs[-1])
    # slot index of each origin: inverse permutation
    slot_of = np.zeros(N, np.int64)
    for r in range(NCORES):
        inv = np.zeros(NPC, np.int64)
        inv[perms[r]] = np.arange(NPC)
        slot_of[r * NPC:(r + 1) * NPC] = inv
    okey = slot_of[origin] + (origin // NPC) * NPC  # global slot order key
    order = np.argsort(okey, kind="stable")
    o_s, d_s = origin[order], dest[order]
    s_s = slot_of[o_s]  # slot within core (0..NPC-1)
    core_of = o_s // NPC
    wloc = s_s // WIN
    percore = []
    for r in range(NCORES):
        dd_ = np.zeros(NB, np.int16)  # global dest node id
        ol = np.zeros(NB, np.int64)  # origin-local slot (0..127); pads 0
        vm = np.zeros(NB, bool)  # valid (non-pad)
        et = np.zeros((NB, H), np.float32)
        cm = core_of == r
        for w in range(W):
            m = cm & (wloc == w)
            cnt = int(m.sum())
            o = int(cofs[w])
            dperm = (d_s[m] // NPC) * NPC + slot_of[d_s[m]]
            dd_[o:o + cnt] = dperm.astype(np.int16)
            ol[o:o + cnt] = s_s[m] - w * WIN
            vm[o:o + cnt] = True
            et[o:o + cnt] = ew[order[m]]
        percore.append(dict(dd=dd_, ol=ol, vm=vm, et=et))
    return percore, Tw, cofs, perms


def _wrap_idx(idx_flat):
    """int16 [n] -> wrapped [128, n/16] layout for dma_gather (idx i at
    [i%16, i//16], replicated over the 8 Q7 partition groups)."""
    w = idx_flat.reshape(-1, 16).T  # [16, n/16]
    return np.tile(w, (8, 1)).astype(np.int16)


def _win_geometry(Tw):
    """Per-window halves (gather granularity) and 4-tile blocks."""
    geo = []
    for T in Tw:
        hn0 = min(T, ((T + 1) // 2 + 3) // 4 * 4)
        halves = [(0, hn0)] + ([(hn0, T - hn0)] if T > hn0 else [])
        blocks = []
        for h0, hn in halves:
            for b0 in range(h0, h0 + hn, 4):
                blocks.append((b0, min(4, h0 + hn - b0)))
        geo.append((halves, blocks))
    return geo


def _build_program(Tw, mock_ag=False, use_bias=True):
    import concourse.bass as bass
    import concourse.bacc as bacc
    import concourse.mybir as mybir
    import concourse.tile as tile

    dt = mybir.dt
    Alu = mybir.AluOpType
    Act = mybir.ActivationFunctionType

    TM = max(Tw)
    NB = W * TM * ET
    cofs = [w * TM * ET for w in range(W + 1)]
    GEO = _win_geometry(Tw)

    nc = bacc.Bacc(
        "TRN2", target_bir_lowering=False, debug=False, num_devices=NCORES
    )

    # ---------------- I/O ----------------
    xsb_t = nc.dram_tensor("xsb", [W * 128, D], dt.bfloat16, kind="ExternalInput")
    wq_t = nc.dram_tensor("wq", [128, 4, D], dt.bfloat16, kind="ExternalInput")
    wk_t = nc.dram_tensor("wk", [128, 4, D], dt.bfloat16, kind="ExternalInput")
    wv_t = nc.dram_tensor("wv", [128, 4, D], dt.bfloat16, kind="ExternalInput")
    w12_t = nc.dram_tensor("w12", [128, 4, D], dt.bfloat16, kind="ExternalInput")
    bias_t = nc.dram_tensor("bias", [1, 4, D], dt.bfloat16, kind="ExternalInput")
    hmask_t = nc.dram_tensor("hmask", [128, 4, H], dt.bfloat16, kind="ExternalInput")
    m1w_t = nc.dram_tensor("m1w", [H, D], dt.bfloat16, kind="ExternalInput")
    ones_t = nc.dram_tensor("ones1", [1, 128], dt.bfloat16, kind="ExternalInput")
    ident_t = nc.dram_tensor("ident", [128, 128], dt.bfloat16, kind="ExternalInput")
    ident8_t = nc.dram_tensor("ident8", [128, 128], dt.float8e4, kind="ExternalInput")
    dw_t = nc.dram_tensor("dw", [128, NB // 16], dt.int16, kind="ExternalInput")
    st_t = nc.dram_tensor("st", [128, NB], dt.bfloat16, kind="ExternalInput")
    stt_t = nc.dram_tensor("stt", [128, NB], dt.float8e4, kind="ExternalInput")
    ewt_t = nc.dram_tensor("ewt", [H, NB], dt.bfloat16, kind="ExternalInput")
    out_t = nc.dram_tensor("out", [W * 128, D], dt.bfloat16, kind="ExternalOutput")

    with tile.TileContext(nc) as tc:
        with (
            tc.tile_pool(name="const", bufs=1) as cpool,
            tc.tile_pool(name="persist", bufs=1) as ppool,
            tc.tile_pool(name="dram", bufs=1, space="DRAM") as dpool,
        ):
            # persistent x: first two windows loaded before anything else so
            # LayerNorm starts immediately; critical consts next; rest after.
            xg_all0 = ppool.tile([128, W, D], dt.bfloat16)
            for g in range(2):
                nc.sync.dma_start(
                    xg_all0[:, g, :], xsb_t.ap()[g * 128:(g + 1) * 128, :]
                )

            # constants
            wq = cpool.tile([128, 4, D], dt.bfloat16)
            wk = cpool.tile([128, 4, D], dt.bfloat16)
            wv = cpool.tile([128, 4, D], dt.bfloat16)
            w12 = cpool.tile([128, 4, D], dt.bfloat16)
            biases = cpool.tile([1, 4, D], dt.bfloat16)
            hmask = cpool.tile([128, 4, H], dt.bfloat16)
            m1w = cpool.tile([H, D], dt.bfloat16)
            ones1 = cpool.tile([1, 128], dt.bfloat16)
            ident = cpool.tile([128, 128], dt.bfloat16)
            ident8 = cpool.tile([128, 128], dt.float8e4)
            dw = cpool.tile([128, NB // 16], dt.int16)
            for tl, tn in [(ident, ident_t), (wk, wk_t), (wv, wv_t)]:
                nc.sync.dma_start(tl[:], tn.ap())
            for g in range(2, W):
                nc.sync.dma_start(
                    xg_all0[:, g, :], xsb_t.ap()[g * 128:(g + 1) * 128, :]
                )
            for tl, tn in [
                (wq, wq_t), (w12, w12_t),
                (biases, bias_t), (hmask, hmask_t), (m1w, m1w_t),
                (ones1, ones_t), (ident8, ident8_t), (dw, dw_t),
            ]:
                nc.sync.dma_start(tl[:], tn.ap())

            # persistent activations
            q_sb = ppool.tile([128, W, D], dt.bfloat16)
            xg_all = xg_all0
            zT_all = ppool.tile([128, W, 4, 128], dt.bfloat16)
            values = ppool.tile([128, W, D], dt.bfloat16)
            vT = ppool.tile([128, 4, W, 128], dt.bfloat16)

            # collective buffers (Shared: all 8 cores on one device's HBM)
            k_in = dpool.tile([NPC, D], dt.bfloat16)
            v_in = dpool.tile([NPC, D], dt.bfloat16)
            k_full = dpool.tile([N, D], dt.bfloat16, addr_space="Shared")
            v_full = dpool.tile([N, D], dt.bfloat16, addr_space="Shared")

            # ---------------- Phase A: LN + QKV ----------------
            pb_ctx = tc.tile_pool(name="pB", bufs=2)
            pb = pb_ctx.__enter__()
            TMET = TM * ET

            def loadw(w):
                co = cofs[w]
                T = Tw[w]
                stw = pb.tile([128, TMET], dt.bfloat16, tag="stw")
                nc.sync.dma_start(stw[:, :T * ET], st_t.ap()[:, co:co + T * ET])
                sttw = pb.tile([128, TMET], dt.float8e4, tag="sttw")
                nc.sync.dma_start(sttw[:, :T * ET], stt_t.ap()[:, co:co + T * ET])
                ewtw = pb.tile([H, TMET], dt.bfloat16, tag="ewtw")
                nc.sync.dma_start(ewtw[:, :T * ET], ewt_t.ap()[:, co:co + T * ET])
                return stw, sttw, ewtw

            preloaded = [loadw(0), loadw(1)]
            with (
                nc.named_scope("phaseA_qkv"),
                tc.tile_pool(name="pA", bufs=5) as pa,
                tc.tile_pool(name="psA", bufs=3, space="PSUM") as psa,
            ):
                def proj(g, wt, bi, dst_sb, dram=None):
                    lo = g * 128
                    rows = min(128, NPC - lo)
                    ps = psa.tile([128, D], dt.float32, tag="qkv_ps")
                    for c in range(4):
                        nc.tensor.matmul(
                            ps[:], zT_all[:, g, c, :], wt[:, c, :],
                            start=(c == 0), stop=(c == 3 and not use_bias),
                        )
                    if use_bias:
                        nc.tensor.matmul(
                            ps[:], ones1[:], biases[:, bi, :], start=False, stop=True
                        )
                    if dst_sb is None:
                        kvt = pa.tile([128, D], dt.bfloat16, tag="kvt")
                        if dram is v_in:
                            nc.vector.tensor_copy(kvt[:], ps[:])
                        else:
                            nc.scalar.copy(kvt[:], ps[:])
                        nc.sync.dma_start(dram[lo:lo + rows, :], kvt[:rows, :])
                    else:
                        nc.scalar.copy(dst_sb[:], ps[:])

                def kproj(g):
                    proj(g, wk, 1, None, k_in)

                def vproj(g):
                    proj(g, wv, 2, None, v_in)

                for g in range(W):
                    xg = xg_all[:, g, :]
                    musum = pa.tile([128, 1], dt.float32, tag="musum")
                    nc.vector.tensor_reduce(musum[:], xg, mybir.AxisListType.X, Alu.add)
                    mu = pa.tile([128, 1], dt.float32, tag="mu")
                    nc.vector.tensor_scalar_mul(mu[:], musum[:], 1.0 / D)
                    xc = pa.tile([128, D], dt.bfloat16, tag="xc")
                    nc.vector.tensor_scalar(xc[:], xg, mu[:], None, Alu.subtract)
                    sq = pa.tile([128, D], dt.bfloat16, tag="sq")
                    vs = pa.tile([128, 1], dt.float32, tag="vs")
                    nc.vector.scalar_tensor_tensor(
                        sq[:], xc[:], 1.0, xc[:], Alu.bypass, Alu.mult, accum_out=vs[:]
                    )
                    vr = pa.tile([128, 1], dt.float32, tag="vr")
                    nc.vector.tensor_scalar(vr[:], vs[:], 1.0 / D, EPS_LN, Alu.mult, Alu.add)
                    sd = pa.tile([128, 1], dt.float32, tag="sd")
                    nc.scalar.sqrt(sd[:], vr[:])
                    rstd = pa.tile([128, 1], dt.float32, tag="rstd")
                    nc.vector.reciprocal(rstd[:], sd[:])
                    z = pa.tile([128, D], dt.bfloat16, tag="z")
                    nc.vector.tensor_scalar(z[:], xc[:], rstd[:], None, Alu.mult)
                    zT_ps = psa.tile([128, 4, 128], dt.bfloat16, tag="zT_ps")
                    for c in range(4):
                        nc.tensor.transpose(
                            zT_ps[:, c, :], z[:, c * 128:(c + 1) * 128], ident[:]
                        )
                    nc.scalar.copy(zT_all[:, g, :, :], zT_ps[:])
                    kproj(g)
                    vproj(g)

                if mock_ag:
                    nc.sync.dma_start(k_full[0:NPC, :], k_in[:])
                else:
                    nc.gpsimd.collective_compute(
                        "AllGather", Alu.bypass,
                        replica_groups=[list(range(NCORES))],
                        ins=[k_in.opt()], outs=[k_full.opt()],
                    )
                if mock_ag:
                    nc.sync.dma_start(v_full[0:NPC, :], v_in[:])
                else:
                    nc.gpsimd.collective_compute(
                        "AllGather", Alu.bypass,
                        replica_groups=[list(range(NCORES))],
                        ins=[v_in.opt()], outs=[v_full.opt()],
                    )
                for g in range(W):
                    proj(g, wq, 0, q_sb[:, g, :])

            # ---------------- Phase B: edge loop ----------------
            with (
                nc.named_scope("phaseB_edges"),
                tc.tile_pool(name="psSel", bufs=1, space="PSUM") as pssel,
                tc.tile_pool(name="psSc", bufs=1, space="PSUM") as pssc,
                tc.tile_pool(name="psAcc", bufs=2, space="PSUM") as psacc,
                tc.tile_pool(name="psDen", bufs=1, space="PSUM") as psden,
            ):
                ghalves = _win_geometry([TM])[0][0]
                wctx = {}

                def setup_w(w):
                    T = Tw[w]
                    co = cofs[w]
                    gh = []
                    for h0, hn in ghalves:
                        ni = hn * ET
                        c0 = (co + h0 * ET) // 16
                        kT = pb.tile([128, 4, ni], dt.bfloat16, tag=f"kT{h0 > 0}")
                        nc.gpsimd.dma_gather(
                            out_ap=kT[:], in_ap=k_full[:],
                            idxs_ap=dw[:, c0:c0 + ni // 16],
                            num_idxs=ni, num_idxs_reg=ni, elem_size=D,
                            transpose=True, single_packet=False,
                        )
                        vG = pb.tile([128, hn, D], dt.bfloat16, tag=f"vG{h0 > 0}")
                        nc.gpsimd.dma_gather(
                            out_ap=vG[:], in_ap=v_full[:],
                            idxs_ap=dw[:, c0:c0 + ni // 16],
                            num_idxs=ni, num_idxs_reg=ni, elem_size=D,
                            single_packet=False,
                        )
                        gh.append((kT, vG))
                    stw, sttw, ewtw = preloaded[w] if w < 2 else loadw(w)
                    wctx[w] = dict(gh=gh, stw=stw, sttw=sttw, ewtw=ewtw)

                def s1(w, t0, bt):
                    if w not in wctx:
                        setup_w(w)
                    cx = wctx[w]
                    EB = bt * ET
                    ecol = t0 * ET
                    hf = 0 if t0 < ghalves[0][1] else 1
                    kT, vG = cx["gh"][hf]
                    h0 = ghalves[hf][0]
                    qgT = pssel.tile([128, 4, 512], dt.bfloat16, tag="qgT")
                    for c in range(4):
                        nc.tensor.transpose(
                            qgT[:, c, :EB],
                            q_sb[:, w, c * 128:(c + 1) * 128],
                            cx["stw"][:, ecol:ecol + EB],
                        )
                    kq = pb.tile([128, 4, 512], dt.bfloat16, tag="kq")
                    nc.vector.tensor_tensor(
                        kq[:, :, :EB], kT[:, :, (t0 - h0) * ET:(t0 - h0) * ET + EB],
                        qgT[:, :, :EB], Alu.mult,
                    )
                    return dict(w=w, t0=t0, bt=bt, kq=kq, vG=vG, h0=h0)

                def s2(it):
                    w, t0, bt, kq = it["w"], it["t0"], it["bt"], it["kq"]
                    cx = wctx[w]
                    EB = bt * ET
                    ecol = t0 * ET
                    sc = pssc.tile([8, 512], dt.float32, tag="sc")
                    for c in range(4):
                        nc.tensor.matmul(
                            sc[:, :EB], hmask[:, c, :], kq[:, c, :EB],
                            start=(c == 0), stop=(c == 3),
                        )
                    ws = pb.tile([8, 512], dt.bfloat16, tag="ws")
                    nc.vector.tensor_tensor(
                        ws[:, :EB], sc[:, :EB], cx["ewtw"][:, ecol:ecol + EB], Alu.mult
                    )
                    ews = pb.tile([8, 512], dt.bfloat16, tag="ews")
                    nc.scalar.activation(ews[:, :EB], ws[:, :EB], Act.Exp)
                    it["ews"] = ews
                    return it

                def s3a(it):
                    w, t0, bt, ews = it["w"], it["t0"], it["bt"], it["ews"]
                    vG, h0 = it["vG"], it["h0"]
                    b_ps = pssel.tile([128, 4, D], dt.bfloat16, tag="b_ps")
                    for t in range(bt):
                        nc.tensor.transpose(
                            b_ps[:, t, :], ews[:, t * ET:(t + 1) * ET], m1w[:]
                        )
                    wvx = pb.tile([128, 4, D + H], dt.bfloat16, tag="wvx")
                    nc.vector.tensor_tensor(
                        wvx[:, :bt, :D], vG[:, t0 - h0:t0 - h0 + bt, :],
                        b_ps[:, :bt, :D], Alu.mult,
                    )
                    nc.scalar.copy(wvx[:, :bt, D:D + H], b_ps[:, :bt, ::HD])
                    it["wvx"] = wvx
                    return it

                def s4(it):
                    w, t0, bt, wvx = it["w"], it["t0"], it["bt"], it["wvx"]
                    cx = wctx[w]
                    T = Tw[w]
                    if t0 == 0:
                        unnorm_t = psacc.tile([128, D], dt.float32, tag="unnorm")
                        denomB_t = psden.tile([128, H], dt.float32, tag="denomB")
                        cx["unnorm"], cx["denomB"] = unnorm_t, denomB_t
                    unnorm, denomB = cx["unnorm"], cx["denomB"]
                    for t in range(bt):
                        tt = t0 + t
                        stcol = tt * ET
                        nc.tensor.matmul(
                            unnorm[:], cx["sttw"][:, stcol:stcol + ET], wvx[:, t, :D],
                            start=(tt == 0), stop=(tt == T - 1),
                        )
                        nc.tensor.matmul(
                            denomB[:], cx["sttw"][:, stcol:stcol + ET], wvx[:, t, D:D + H],
                            start=(tt == 0), stop=(tt == T - 1),
                        )
                    if t0 + bt == T:
                        epi1(w, unnorm, denomB)
                        epi_q.append([w, 0])
                        del wctx[w]

                def epi1(w, unnorm, denomB):
                    den8 = pb.tile([128, H], dt.float32, tag="den8")
                    nc.vector.tensor_scalar(den8[:], denomB[:], EPS_DEN, None, Alu.add)
                    rec8 = pb.tile([128, H], dt.float32, tag="rec8")
                    nc.vector.reciprocal(rec8[:], den8[:])
                    un_sb = pb.tile([128, D], dt.float32, tag="un_sb")
                    nc.scalar.copy(un_sb[:], unnorm[:])
                    for h in range(H):
                        nc.gpsimd.tensor_scalar(
                            values[:, w, h * HD:(h + 1) * HD],
                            un_sb[:, h * HD:(h + 1) * HD],
                            rec8[:, h:h + 1], None, Alu.mult,
                        )

                def epi2a(w):
                    vt_ps = pssel.tile([128, 4, 512], dt.bfloat16, tag="qgT")
                    for c in range(4):
                        nc.tensor.transpose(
                            vt_ps[:, c, :128], values[:, w, c * 128:(c + 1) * 128], ident[:]
                        )
                    nc.vector.tensor_copy(vT[:, :, w, :], vt_ps[:, :, :128])

                def epi2b(w):
                    mlp_ps = psacc.tile([128, D], dt.float32, tag="unnorm")
                    for c in range(4):
                        nc.tensor.matmul(
                            mlp_ps[:], vT[:, c, w, :], w12[:, c, :],
                            start=(c == 0), stop=(c == 3 and not use_bias),
                        )
                    if use_bias:
                        nc.tensor.matmul(
                            mlp_ps[:], ones1[:], biases[:, 3, :], start=False, stop=True
                        )
                    og = pb.tile([128, D], dt.bfloat16, tag="og")
                    nc.vector.tensor_tensor(og[:], mlp_ps[:], xg_all[:, w, :], Alu.add)
                    nc.sync.dma_start(out_t.ap()[w * 128:(w + 1) * 128, :], og[:])

                stream = [
                    (w, t0, bt) for w in range(W) for (t0, bt) in GEO[w][1]
                ]
                epi_q = []
                q1 = q2 = q3 = None
                for item in stream + [None, None, None]:
                    nxt = s1(*item) if item is not None else None
                    if q1 is not None:
                        q1 = s2(q1)
                    if q2 is not None:
                        q2 = s3a(q2)
                    if q3 is not None:
                        s4(q3)
                    for e in epi_q:
                        e[1] += 1
                    for e in epi_q:
                        if e[1] == 4:
                            epi2a(e[0])
                    while epi_q and epi_q[0][1] >= 6:
                        epi2b(epi_q.pop(0)[0])
                    q3 = q2
                    q2 = q1
                    q1 = nxt
                for e in epi_q:
                    if e[1] < 4:
                        epi2a(e[0])
                    epi2b(e[0])

            pb_ctx.__exit__(None, None, None)

    nc.compile()
    from concourse.bass_interp import get_hw_module

    nc.m = get_hw_module(nc.m)
    return nc


def kernel(x, edge_index, edge_weights, ln_g, ln_b, Wq, bq, Wk, bk, Wv, bv,
           W1, b1, W2, b2, _trace=False):
    x = np.asarray(x, np.float32)
    ei = np.asarray(edge_index)
    ew = np.asarray(edge_weights, np.float32)
    origin, dest = ei[0].astype(np.int64), ei[1].astype(np.int64)

    percore, Tw, cofs, perms = _host_prep(origin, dest, ew)

    # fold LN affine + attention scale into weights (host, fp32)
    ln_g = np.asarray(ln_g, np.float32)
    ln_b = np.asarray(ln_b, np.float32)
    Wq_f = (ln_g[:, None] * np.asarray(Wq, np.float32)) * SCALE
    bq_f = (ln_b @ np.asarray(Wq, np.float32)) * SCALE + np.asarray(bq, np.float32) * SCALE
    Wk_f = ln_g[:, None] * np.asarray(Wk, np.float32)
    bk_f = ln_b @ np.asarray(Wk, np.float32) + np.asarray(bk, np.float32)
    Wv_f = ln_g[:, None] * np.asarray(Wv, np.float32)
    bv_f = ln_b @ np.asarray(Wv, np.float32) + np.asarray(bv, np.float32)
    W12 = np.asarray(W1, np.float32) @ np.asarray(W2, np.float32)
    b12 = np.asarray(b1, np.float32) @ np.asarray(W2, np.float32) + np.asarray(b2, np.float32)

    f8 = ml_dtypes.float8_e4m3

    def chunked(wm):  # [512, 512] -> [128, 4, 512] bf16
        return np.ascontiguousarray(
            wm.reshape(4, 128, D).transpose(1, 0, 2)
        ).astype(bf16)

    def chunked8(wm):  # [512, 512] -> [128, 2, 2, 512] fp8 DoubleRow layout
        return np.ascontiguousarray(
            wm.reshape(2, 2, 128, D).transpose(2, 0, 1, 3)
        ).astype(f8)

    hmask = np.zeros((128, 4, H), np.float32)
    for c in range(4):
        for d in range(128):
            hmask[d, c, (128 * c + d) // HD] = 1.0
    m1w = np.zeros((H, D), np.float32)
    for h in range(H):
        m1w[h, h * HD:(h + 1) * HD] = 1.0
    bias_all = np.stack([bq_f, bk_f, bv_f, b12])[None]  # [1, 4, 512]

    common = dict(
        wq=chunked(Wq_f), wk=chunked(Wk_f), wv=chunked(Wv_f), w12=chunked(W12),
        bias=bias_all.astype(bf16), hmask=hmask.astype(bf16), m1w=m1w.astype(bf16),
        ones1=np.ones((1, 128), bf16),
        ident=np.eye(128, dtype=bf16),
        ident8=np.eye(128, dtype=ml_dtypes.float8_e4m3),
    )

    NB = int(cofs[-1])
    in_maps = []
    ar = np.arange(NB)
    tile_of = ar // ET
    e_in_tile = ar % ET
    for r in range(NCORES):
        pc = percore[r]
        ol = pc["ol"]
        vm = pc["vm"]
        st = np.zeros((128, NB), bf16)
        st[ol, ar] = 1
        stt = np.zeros((128, NB), ml_dtypes.float8_e4m3)
        stt[e_in_tile[vm], tile_of[vm] * ET + ol[vm]] = 1
        xsb = np.zeros((W * 128, D), np.float32)
        xsb[:NPC] = x[r * NPC:(r + 1) * NPC][perms[r]]
        in_maps.append(dict(
            xsb=xsb.astype(bf16),
            xs=xsb,
            dw=_wrap_idx(pc["dd"]),
            st=st, stt=stt,
            ewt=np.ascontiguousarray(pc["et"].T).astype(bf16),
            **common,
        ))

    use_bias = any(
        float(np.abs(b).max()) > 0 for b in (bq_f, bk_f, bv_f, b12)
    )
    nc = _build_program(Tw, use_bias=use_bias)
    from concourse import bass_utils

    res = bass_utils.run_bass_kernel_spmd(
        nc, in_maps, core_ids=list(range(NCORES))
    )
    outs = []
    for r in range(NCORES):
        orows = res.results[r]["out"][:NPC]
        unperm = np.empty_like(orows)
        unperm[perms[r]] = orows
        outs.append(unperm)
    out = np.concatenate(outs, axis=0)
    kernel.last_result = res
    if _trace:
        import bench_hw

        kernel.exec_time_ns = bench_hw.bench(nc, in_maps, NCORES)
    return out.astype(np.float32)
